# revision 1
# baseline (speedup 1.0000x reference)
"""Multi-head causal attention (B=1024, T=64, C=768, H=12, D=64) on 8 TRN2
NeuronCores, data-parallel over the batch dimension (128 batches/core).

Dataflow per core (all matmuls bf16, fp32 PSUM accumulate):
  - X [8192, 768] is loaded, cast to bf16 and PE-transposed into
    XT [c, tokens] chunks.
  - QT/KT [hd, tok] = WT.T @ XT (weights stationary); V is produced in natural
    [tok, hd] layout (XT stationary, WvT moving) and stored with an extra
    ones column per head ("Vaug") so the attention denominator falls out of
    the AV matmul for free.
  - Per (batch, head): scoresT[s,t] = KT.T @ QT; Pexp = exp(scores/8) * causal
    mask; Y[t, (h,d)|den] = PexpT.T @ Vaug; normalize with the reciprocal of
    the den column.
  - Y is PE-transposed to YT [hd, t]; out[t, c] = YT.T @ WpT + bp.
"""

import numpy as np

P = 128
B, T, C, H, Dh = 1024, 64, 768, 12, 64
HD = H * Dh            # 768
NCC = C // P           # 6 contraction chunks
NHD = HD // P          # 6 hd chunks
N_CORES = 8

_cache = {}


def _patch_tile_drain(tile, mybir):
    """walrus CTRL (Drain) ops in this toolchain accept only 1 sem-wait;
    spread the TileContext exit-drain's waits across preceding SP nops."""
    from concourse.vector_clock import ScopedClock

    if getattr(tile.TileContext, "_drain_patched", False):
        return

    def _drain_and_barrier(self, tick_clock, wait_clock):
        nc = self.nc
        drain_inst = nc.sync.drain()
        wait_clock.add_sem_waits(
            drain_inst.ins, ScopedClock({None: tick_clock.global_clock})
        )
        waits = list(drain_inst.ins.sync_info.on_wait)
        if len(waits) > 1:
            drain_inst.ins.sync_info.on_wait = waits[-1:]
            cur_bb = nc.cur_bb.bb
            idx = cur_bb.instructions.index(drain_inst.ins)
            extra = []
            for w in waits[:-1]:
                nop = mybir.InstNoOp(name=f"I-{nc.next_id()}", ins=[], outs=[])
                nop.engine = drain_inst.ins.engine
                nop.sync_info = mybir.SyncInfo(on_wait=[w], on_update=[])
                nc.register_instruction(nop)
                extra.append(nop)
            cur_bb.instructions[idx:idx] = extra
        nc.all_engine_barrier()
        assert self.sems is not None
        popped = nc._tile_sem_poison_stack.pop()
        assert popped is self._sem_poison
        nc.clear_and_free_semaphores(list(self.sems.allocated().values()))
        nc.all_engine_barrier()

    tile.TileContext._drain_and_barrier = _drain_and_barrier
    tile.TileContext._drain_patched = True


def _install_loud_cc_hook():
    """Surface real exceptions from the neuronx_cc hook (C wrapper eats them)."""
    from concourse import bass2jax as _b2j
    if getattr(_b2j, "_loud_hook_installed", False):
        return
    _orig = _b2j.neuronx_cc_hook
    def _loud(*a, **k):
        try:
            return _orig(*a, **k)
        except BaseException:
            import traceback
            traceback.print_exc()
            raise
    _b2j.neuronx_cc_hook = _loud
    _b2j._loud_hook_installed = True


def _split_multi_waits(nc, mybir, K=1):
    """This walrus build supports only one sem-wait per instruction: move
    excess waits onto same-engine NOPs inserted directly before the owner."""
    def fix_block(bb):
        insts = bb.instructions
        i = 0
        while i < len(insts):
            ins = insts[i]
            si = ins.sync_info
            w = list(si.on_wait) if si is not None and si.on_wait else []
            if len(w) > K:
                carriers = []
                for j in range(0, len(w) - K, K):
                    nop = mybir.InstNoOp(name=f"I-{nc.next_id()}", ins=[], outs=[])
                    nop.engine = ins.engine
                    nop.sync_info = mybir.SyncInfo(on_wait=w[j:j + K], on_update=[])
                    nc.register_instruction(nop)
                    carriers.append(nop)
                si.on_wait = w[len(w) - K:]
                insts[i:i] = carriers
                i += len(carriers)
            i += 1
    for fn in nc.m.functions:
        for bb in fn.blocks:
            fix_block(bb)


def _bp_bcast_ap(bass, bp_d):
    a = bp_d[:]
    return bass.AP(tensor=a.tensor, offset=a.offset, ap=[[0, P]] + list(a.ap))


def build_nc(B_loc=B // N_CORES, chunk_tok=2048, phases=6):
    import concourse.bass as bass
    import concourse.tile as tile
    from concourse import mybir
    from contextlib import ExitStack

    _patch_tile_drain(tile, mybir)
    _install_loud_cc_hook()

    F32 = mybir.dt.float32
    BF16 = mybir.dt.bfloat16
    AF = mybir.ActivationFunctionType
    ALU = mybir.AluOpType

    BT = B_loc * T
    chunk_tok = min(chunk_tok, BT)
    n_chunks = BT // chunk_tok
    assert n_chunks * chunk_tok == BT
    TT = chunk_tok // P     # 128-token tiles per chunk
    NB = chunk_tok // T     # batches per chunk

    nc = bass.Bass()
    x_d = nc.declare_dram_parameter("x", [B_loc, T, C], F32, isOutput=False)
    wq_d = nc.declare_dram_parameter("Wq", [H, Dh, C], F32, isOutput=False)
    wk_d = nc.declare_dram_parameter("Wk", [H, Dh, C], F32, isOutput=False)
    wv_d = nc.declare_dram_parameter("Wv", [H, Dh, C], F32, isOutput=False)
    wp_d = nc.declare_dram_parameter("Wp", [C, HD], F32, isOutput=False)
    bp_d = nc.declare_dram_parameter("bp", [C], F32, isOutput=False)
    id_d = nc.declare_dram_parameter("ident", [P, P], BF16, isOutput=False)
    mk_d = nc.declare_dram_parameter("mask", [P, T], BF16, isOutput=False)
    out_d = nc.declare_dram_parameter("out", [B_loc, T, C], F32, isOutput=True)

    xf = x_d[:].flatten_outer_dims()      # [BT, C]
    of = out_d[:].flatten_outer_dims()    # [BT, C]

    with tile.TileContext(nc) as tc, ExitStack() as ctx:
        sing = ctx.enter_context(tc.tile_pool(name="sing", bufs=1))
        stage = ctx.enter_context(tc.tile_pool(name="stage", bufs=3))
        ostage = ctx.enter_context(tc.tile_pool(name="ostage", bufs=3))
        chunkp = ctx.enter_context(tc.tile_pool(name="chunkp", bufs=1))
        pexp_p = ctx.enter_context(tc.tile_pool(name="pexp", bufs=4))
        y_p = ctx.enter_context(tc.tile_pool(name="y", bufs=3))
        yt_p = ctx.enter_context(tc.tile_pool(name="yt", bufs=3))
        small = ctx.enter_context(tc.tile_pool(name="small", bufs=6))
        pp = ctx.enter_context(tc.tile_pool(name="pp", bufs=8, space="PSUM"))

        def ptile(pdim, shape, name, dt=None):
            # all PSUM tiles share one 1-bank slot class
            t = pp.tile([P, 512], dt or F32, tag="ps", name=name)
            flat = t[:pdim, : int(np.prod(shape[1:]))]
            return flat.rearrange(
                "p (a b) -> p a b", a=shape[1]
            ) if len(shape) == 3 else flat

        # ---- constants ----
        id_sb = sing.tile([P, P], BF16)
        nc.sync.dma_start(out=id_sb, in_=id_d[:])
        mask_sb = sing.tile([P, T], BF16)
        nc.sync.dma_start(out=mask_sb, in_=mk_d[:])
        bp_bc = sing.tile([P, C], F32)
        nc.gpsimd.dma_start(
            out=bp_bc,
            in_=_bp_bcast_ap(bass, bp_d),
        )

        # ---- persistent block-diagonal attention operands ----
        NBmax = chunk_tok // T
        ktbd = sing.tile([P, NHD, NBmax, P], BF16, name="ktbd")
        nc.gpsimd.memset(ktbd, 0.0)
        vbd = sing.tile([P, NHD, NBmax, 2 * (Dh + 1)], BF16, name="vbd")
        nc.gpsimd.memset(vbd, 0.0)
        nc.gpsimd.memset(vbd[0:T, :, :, Dh:Dh + 1], 1.0)
        nc.gpsimd.memset(vbd[T:P, :, :, 2 * Dh + 1:2 * Dh + 2], 1.0)

        # ---- weight prep: WT_sb[col_in, col_out, row] = flat[row, col]^T ----
        def prep_wT(wflat, name):
            wT = sing.tile([P, NCC, 768], BF16, name=name)
            for r in range(6):
                wrow = stage.tile([P, 768], F32, tag="wstage")
                nc.sync.dma_start(out=wrow, in_=wflat[r * P:(r + 1) * P, :])
                wbf = stage.tile([P, 768], BF16, tag="wbf")
                nc.scalar.copy(out=wbf, in_=wrow)
                psA = ptile(P, (P, 4, P), f"{name}_psA_{r}", BF16)
                psB = ptile(P, (P, 2, P), f"{name}_psB_{r}", BF16)
                for j in range(6):
                    dst = psA[:, j, :] if j < 4 else psB[:, j - 4, :]
                    nc.tensor.transpose(dst, wbf[:, j * P:(j + 1) * P], id_sb)
                nc.vector.tensor_copy(
                    out=wT[:, 0:4, r * P:(r + 1) * P], in_=psA)
                nc.vector.tensor_copy(
                    out=wT[:, 4:6, r * P:(r + 1) * P], in_=psB)
            return wT

        wqT = prep_wT(wq_d[:].flatten_outer_dims(), "wqT")
        wkT = prep_wT(wk_d[:].flatten_outer_dims(), "wkT")
        wvT = prep_wT(wv_d[:].flatten_outer_dims(), "wvT")
        wpT = prep_wT(wp_d[:], "wpT")

        for ci in range(n_chunks):
            tok0 = ci * chunk_tok

            # ---- P0: load + cast + transpose X ----
            xT = chunkp.tile([P, NCC, chunk_tok], BF16, tag="xT")
            for it in range(TT):
                row0 = tok0 + it * P
                xr = stage.tile([P, C], F32, tag="xstage")
                nc.sync.dma_start(out=xr, in_=xf[row0:row0 + P, :])
                xb = stage.tile([P, C], BF16, tag="xbf")
                nc.scalar.copy(out=xb, in_=xr)
                psA = ptile(P, (P, 4, P), "x_psA", BF16)
                psB = ptile(P, (P, 2, P), "x_psB", BF16)
                for j in range(6):
                    dst = psA[:, j, :] if j < 4 else psB[:, j - 4, :]
                    nc.tensor.transpose(dst, xb[:, j * P:(j + 1) * P], id_sb)
                nc.scalar.copy(out=xT[:, 0:4, it * P:(it + 1) * P], in_=psA)
                nc.scalar.copy(out=xT[:, 4:6, it * P:(it + 1) * P], in_=psB)

            if phases < 2:
                continue
            # ---- P1a: QT / KT projections (weights stationary) ----
            SUBW = min(512, chunk_tok)
            n_sub = chunk_tok // SUBW
            qT = chunkp.tile([P, NHD, chunk_tok], BF16, tag="qT")
            nbsub = SUBW // T          # batches per SUBW token block
            for wT, dst in ((wqT, "q"), (wkT, "k")):
                for m in range(NHD):
                    pss = [ptile(P, (P, SUBW), f"proj_{m}_{s}") for s in range(n_sub)]
                    for cc in range(NCC):
                        lhs = wT[:, cc, m * P:(m + 1) * P]
                        for s in range(n_sub):
                            nc.tensor.matmul(
                                pss[s], lhs,
                                xT[:, cc, s * SUBW:(s + 1) * SUBW],
                                start=(cc == 0), stop=(cc == NCC - 1))
                    for s in range(n_sub):
                        if dst == "q":
                            nc.vector.tensor_copy(
                                out=qT[:, m, s * SUBW:(s + 1) * SUBW], in_=pss[s])
                        else:
                            b0 = s * nbsub
                            nc.vector.tensor_copy(
                                out=ktbd[0:T, m, b0:b0 + nbsub, 0:T],
                                in_=pss[s][0:T].rearrange(
                                    "p (nb t) -> p nb t", nb=nbsub))
                            nc.vector.tensor_copy(
                                out=ktbd[T:P, m, b0:b0 + nbsub, T:P],
                                in_=pss[s][T:P].rearrange(
                                    "p (nb t) -> p nb t", nb=nbsub))
            if phases < 3:
                continue
            # ---- P1b: V in natural layout, then block-diag via remap DMAs ----
            v_sb = chunkp.tile([P, TT, H, Dh], BF16, tag="v_sb")
            for it in range(TT):
                psA = ptile(P, (P, 512), "v_psA")
                psB = ptile(P, (P, 256), "v_psB")
                for cc in range(NCC):
                    lhs = xT[:, cc, it * P:(it + 1) * P]
                    nc.tensor.matmul(psA, lhs, wvT[:, cc, 0:512],
                                     start=(cc == 0), stop=(cc == NCC - 1))
                    nc.tensor.matmul(psB, lhs, wvT[:, cc, 512:768],
                                     start=(cc == 0), stop=(cc == NCC - 1))
                nc.scalar.copy(
                    out=v_sb[:, it, 0:8, :],
                    in_=psA.rearrange("p (a b) -> p a b", a=8))
                nc.scalar.copy(
                    out=v_sb[:, it, 8:12, :],
                    in_=psB.rearrange("p (a b) -> p a b", a=4))
            vbd_v = vbd.rearrange("p a (nb2 two) c -> p a nb2 two c", two=2)
            for p_ in range(NHD):
                for par in range(2):
                    nc.sync.dma_start(
                        out=vbd_v[0:T, p_, :, par, 0:Dh],
                        in_=v_sb[par * T:(par + 1) * T, :, 2 * p_, :])
                    nc.sync.dma_start(
                        out=vbd_v[T:P, p_, :, par, Dh + 1:2 * Dh + 1],
                        in_=v_sb[par * T:(par + 1) * T, :, 2 * p_ + 1, :])
            if phases < 4:
                continue
            # ---- P2+P3: attention, Y transpose, output projection ----
            for it in range(TT):
                yb = y_p.tile([P, HD], BF16, tag="yb")
                y_ps = [ptile(P, (P, 3, 2 * (Dh + 1)), f"y_ps{h2}") for h2 in range(2)]
                for half in range(2):          # two batches per 128-token tile
                    b = it * 2 + half
                    prow = half * T
                    bt0 = b * T
                    s_ps = ptile(P, (P, NHD, T), f"s_ps{half}")
                    for p_ in range(NHD):
                        nc.tensor.matmul(
                            s_ps[:, p_, :],
                            ktbd[:, p_, b, :],
                            qT[:, p_, bt0:bt0 + T],
                            start=True, stop=True)
                    pex = pexp_p.tile([P, NHD, T], BF16, tag="pex", name="pex")
                    nc.scalar.activation(
                        out=pex, in_=s_ps, func=AF.Exp, scale=0.125)
                    nc.vector.tensor_tensor(
                        pex, pex,
                        mask_sb[:, None, :].to_broadcast([P, NHD, T]),
                        ALU.mult)
                    for p_ in range(NHD):
                        nc.tensor.matmul(
                            y_ps[p_ // 3][prow:prow + T, p_ % 3, :],
                            pex[:, p_, :],
                            vbd[:, p_, b, :],
                            start=True, stop=True)
                for h2 in range(2):
                    y_v = y_ps[h2].rearrange("p a (two c) -> p a two c", c=Dh + 1)
                    rec = small.tile([P, 3, 2, 1], F32, tag="rec", name="rec")
                    nc.vector.reciprocal(out=rec, in_=y_v[:, :, :, Dh:Dh + 1])
                    nc.vector.tensor_tensor(
                        yb[:, h2 * 384:(h2 + 1) * 384]
                            .rearrange("p (a two b) -> p a two b", a=3, two=2),
                        y_v[:, :, :, 0:Dh],
                        rec.to_broadcast([P, 3, 2, Dh]),
                        ALU.mult)
                if phases < 5:
                    continue
                # Y transpose
                ytA = ptile(P, (P, 4, P), "yt_psA", BF16)
                ytB = ptile(P, (P, 2, P), "yt_psB", BF16)
                for j in range(6):
                    dst = ytA[:, j, :] if j < 4 else ytB[:, j - 4, :]
                    nc.tensor.transpose(dst, yb[:, j * P:(j + 1) * P], id_sb)
                ytile = yt_p.tile([P, NHD, P], BF16, tag="ytile")
                nc.scalar.copy(out=ytile[:, 0:4, :], in_=ytA)
                nc.scalar.copy(out=ytile[:, 4:6, :], in_=ytB)
                # output projection
                oA = ptile(P, (P, 512), "o_psA")
                oB = ptile(P, (P, 256), "o_psB")
                for j in range(NHD):
                    lhs = ytile[:, j, :]
                    nc.tensor.matmul(oA, lhs, wpT[:, j, 0:512],
                                     start=(j == 0), stop=(j == NHD - 1))
                    nc.tensor.matmul(oB, lhs, wpT[:, j, 512:768],
                                     start=(j == 0), stop=(j == NHD - 1))
                osb = ostage.tile([P, C], F32, tag="osb")
                nc.vector.tensor_tensor(osb[:, 0:512], oA, bp_bc[:, 0:512], ALU.add)
                nc.vector.tensor_tensor(osb[:, 512:768], oB, bp_bc[:, 512:768], ALU.add)
                row0 = tok0 + it * P
                nc.sync.dma_start(out=of[row0:row0 + P, :], in_=osb)

    _split_multi_waits(nc, mybir)
    return nc


def _get_program(B_loc, chunk_tok):
    key = (B_loc, chunk_tok)
    if key not in _cache:
        _cache[key] = build_nc(B_loc, chunk_tok)
    return _cache[key]


def make_const_inputs():
    import ml_dtypes
    ident = np.eye(P, dtype=ml_dtypes.bfloat16)
    # mask[s, t] = 1 if s <= t (causal, scoresT layout)
    m = np.tril(np.ones((T, T), dtype=np.float32)).T.astype(ml_dtypes.bfloat16)
    mask = np.vstack([m, m])   # replicated for both batch partition-halves
    return ident, mask


def prepare(x, Wq, Wk, Wv, Wp, bp, chunk_tok=1024):
    x = np.ascontiguousarray(x, dtype=np.float32)
    B_loc = B // N_CORES
    ident, mask = make_const_inputs()
    nc = _get_program(B_loc, chunk_tok)
    in_maps = []
    for c in range(N_CORES):
        in_maps.append({
            "x": x[c * B_loc:(c + 1) * B_loc],
            "Wq": np.ascontiguousarray(Wq, dtype=np.float32),
            "Wk": np.ascontiguousarray(Wk, dtype=np.float32),
            "Wv": np.ascontiguousarray(Wv, dtype=np.float32),
            "Wp": np.ascontiguousarray(Wp, dtype=np.float32),
            "bp": np.ascontiguousarray(bp, dtype=np.float32),
            "ident": ident,
            "mask": mask,
        })
    return nc, in_maps


def kernel(x, Wq, Wk, Wv, Wp, bp):
    from concourse import bass_utils

    nc, in_maps = prepare(x, Wq, Wk, Wv, Wp, bp)
    res = bass_utils.run_bass_kernel_spmd(nc, in_maps, list(range(N_CORES)))
    return np.concatenate([res.results[c]["out"] for c in range(N_CORES)], axis=0)

